# revision 6
# baseline (speedup 1.0000x reference)
"""2-layer GAT (PyG GATConv, heads=1) on 8 Trainium2 NeuronCores.

Strategy (dst-owner sharding, per spec sharding_hint):
  - Nodes split into 8 contiguous chunks of N/8; edges owned by dst's core.
  - 3 NEFF launches (host does only data movement between them):
    NEFF#1: per-core h1 = embed_chunk @ W1, s1/d1 = h1 @ a_{src,dst}1,
            emitted as bf16 hi/lo split rows -> host concats into a full
            gather table T1 [N, 384]bf16 = [h_hi|h_lo|1.0|s_hi|s_lo|pad].
    NEFF#2: L1 edge phase per core: dma_gather T1 rows by edge src,
            attention weights w_e = exp(LeakyReLU(s_src + d_dst)) computed
            via the separable form  w = max(exp(s)exp(d), exp(.2s)exp(.2d)),
            folded into a per-128-edge-group indicator matrix
            S_alpha[e, c] = w_e * 1[dstcol_e == c]  (one fused DVE op),
            aggregated on the TensorEngine: psum += S_alpha^T @ [h|1].
            The trailing ones column yields the softmax denominator Z free.
            Tail: x2 = relu(out1+b1); h2 = x2 @ W2; s2/d2 -> T2 chunks.
    NEFF#3: same edge machinery on T2 [N, 256]bf16, final sigmoid.
  - Edges are bucketed into <=32768-row source "sets" (dma_gather idx is
    int16) and into 127-dst psum windows; group counts G[set][window] are
    maxed across cores so all 8 cores run one SPMD instruction stream.
"""
import sys

if '/opt/trn_rl_repo' not in sys.path:
    sys.path.insert(0, '/opt/trn_rl_repo')

import numpy as np
import ml_dtypes

from concourse import bacc, mybir
import concourse.tile as tile
from concourse.bass_utils import run_bass_kernel_spmd
from concourse.masks import make_identity

BF16 = ml_dtypes.bfloat16
NCORES = 8
EXEC_NS = []       # per-NEFF exec_time_ns (filled when BASS_TRACE=1)
TRACE_DIRS = []
WIN = 127          # dsts per psum window (col 127 = dummy slot)
MW = 2             # windows per gather megatile
SETROWS = 32768    # int16 gather index range
F32 = mybir.dt.float32
BF = mybir.dt.bfloat16
I16 = mybir.dt.int16
AF = mybir.ActivationFunctionType
OP = mybir.AluOpType


def _trace_kw(tag):
    import os
    if not os.environ.get("BASS_TRACE"):
        return {}
    d = f"/tmp/trace_{tag}"
    os.makedirs(d, exist_ok=True)
    TRACE_DIRS.append(d)
    return {"tmpdir": d}


def _record(res):
    if getattr(res, "exec_time_ns", None) is not None:
        EXEC_NS.append(res.exec_time_ns)


# ----------------------------------------------------------------- host pre
def _preprocess(edge_index, N):
    CH = N // NCORES
    NW = -(-CH // WIN)
    NS = -(-N // SETROWS)
    src = np.concatenate([edge_index[0], np.arange(N, dtype=np.int64)])
    dst = np.concatenate([edge_index[1], np.arange(N, dtype=np.int64)])
    src = src.astype(np.int64)
    dst = dst.astype(np.int64)
    owner = dst // CH
    dl = dst - owner * CH
    sid = src // SETROWS

    cnt = np.zeros((NCORES, NS, NW), np.int64)
    percs = []
    for c in range(NCORES):
        mc = owner == c
        percs.append((src[mc], dl[mc], sid[mc]))
        for s in range(NS):
            ms = percs[c][2] == s
            w = percs[c][1][ms] // WIN
            cnt[c, s] = np.bincount(w, minlength=NW)
    G = -(-cnt.max(axis=0) // 128)          # [NS, NW] groups per (set, window)
    G[cnt.max(axis=0) == 0] = 0

    cumG = np.zeros((NS, NW + 1), np.int64)
    cumG[:, 1:] = np.cumsum(G, axis=1)
    nslot = 128 * cumG[:, -1]               # per-set stream length

    cores = []
    for c in range(NCORES):
        csrc, cdl, csid = percs[c]
        gidx, dcol = [], []
        for s in range(NS):
            ms = csid == s
            esrc, edl = csrc[ms], cdl[ms]
            order = np.argsort(edl, kind='stable')
            esrc, edl = esrc[order], edl[order]
            w = edl // WIN
            col = edl - w * WIN
            # rank within window
            cc = np.zeros(NW + 1, np.int64)
            cc[1:] = np.cumsum(np.bincount(w, minlength=NW))
            rank = np.arange(len(edl)) - cc[w]
            slot = 128 * cumG[s][w] + rank
            arr_i = np.zeros(nslot[s], np.int16)
            arr_c = np.full(nslot[s], 127.0, np.float32)
            arr_i[slot] = (esrc - s * SETROWS).astype(np.int16)
            arr_c[slot] = col
            gi = np.tile(arr_i.reshape(-1, 16).T, (8, 1)) if nslot[s] else \
                np.zeros((128, 0), np.int16)
            dc = arr_c.reshape(-1, 128).T.astype(BF16) if nslot[s] else \
                np.zeros((128, 0), BF16)
            gidx.append(np.ascontiguousarray(gi))
            dcol.append(np.ascontiguousarray(dc))
        cores.append((gidx, dcol))
    return dict(CH=CH, NW=NW, NS=NS, G=G, cumG=cumG, nslot=nslot, cores=cores)


# ------------------------------------------------------------------ NEFF #1
def _build_neff1(N, C, H, CH):
    nc = bacc.Bacc(None, target_bir_lowering=False)
    xT = nc.declare_dram_parameter("xT", [C, CH], F32, isOutput=False)
    W1 = nc.declare_dram_parameter("W1", [C, H], F32, isOutput=False)
    a1s = nc.declare_dram_parameter("a1s", [H, 1], F32, isOutput=False)
    a1d = nc.declare_dram_parameter("a1d", [H, 1], F32, isOutput=False)
    hhi = nc.declare_dram_parameter("hhi", [H, CH], BF, isOutput=True)
    hlo = nc.declare_dram_parameter("hlo", [H, CH], BF, isOutput=True)
    shi = nc.declare_dram_parameter("shi", [1, CH], BF, isOutput=True)
    slo = nc.declare_dram_parameter("slo", [1, CH], BF, isOutput=True)
    d1o = nc.declare_dram_parameter("d1o", [1, CH], F32, isOutput=True)

    KT = -(-C // 128)
    with tile.TileContext(nc) as tc:
        with tc.tile_pool(name="cst", bufs=1) as cp, \
             tc.tile_pool(name="wk", bufs=3) as wp, \
             tc.tile_pool(name="ps", bufs=2, space="PSUM") as pp, \
             tc.tile_pool(name="ps1", bufs=2, space="PSUM") as pp1:
            xts, w1s = [], []
            for k in range(KT):
                kc = min(128, C - 128 * k)
                xt = cp.tile([kc, CH], F32, tag=f"xt{k}")
                nc.sync.dma_start(out=xt[:], in_=xT[128 * k:128 * k + kc, :])
                w1 = cp.tile([kc, H], F32, tag=f"w1{k}")
                nc.sync.dma_start(out=w1[:], in_=W1[128 * k:128 * k + kc, :])
                xts.append(xt)
                w1s.append(w1)
            asb = cp.tile([H, 1], F32, tag="a1s")
            nc.sync.dma_start(out=asb[:], in_=a1s[:])
            adb = cp.tile([H, 1], F32, tag="a1d")
            nc.sync.dma_start(out=adb[:], in_=a1d[:])
            h1T = cp.tile([H, CH], F32, tag="h1T")

            CW = 500
            for o in range(0, CH, CW):
                cw = min(CW, CH - o)
                ph = pp.tile([H, CW], F32, space="PSUM", tag="ph")
                for k in range(KT):
                    nc.tensor.matmul(out=ph[:, :cw], lhsT=w1s[k][:],
                                     rhs=xts[k][:, o:o + cw],
                                     start=(k == 0), stop=(k == KT - 1))
                nc.vector.tensor_copy(out=h1T[:, o:o + cw], in_=ph[:, :cw])
                hh = wp.tile([H, CW], BF, tag="hh")
                nc.scalar.activation(hh[:, :cw], ph[:, :cw], AF.Copy)
                tmp = wp.tile([H, CW], F32, tag="tmp")
                nc.vector.tensor_tensor(out=tmp[:, :cw], in0=ph[:, :cw],
                                        in1=hh[:, :cw], op=OP.subtract)
                hl = wp.tile([H, CW], BF, tag="hl")
                nc.vector.tensor_copy(out=hl[:, :cw], in_=tmp[:, :cw])
                nc.sync.dma_start(out=hhi[:, o:o + cw], in_=hh[:, :cw])
                nc.sync.dma_start(out=hlo[:, o:o + cw], in_=hl[:, :cw])
            for o in range(0, CH, CW):
                cw = min(CW, CH - o)
                ps = pp1.tile([1, CW], F32, space="PSUM", tag="psv")
                nc.tensor.matmul(out=ps[:, :cw], lhsT=asb[:],
                                 rhs=h1T[:, o:o + cw], start=True, stop=True)
                sh = wp.tile([1, CW], BF, tag="sh")
                nc.scalar.activation(sh[:, :cw], ps[:, :cw], AF.Copy)
                tmp = wp.tile([1, CW], F32, tag="tms")
                nc.vector.tensor_tensor(out=tmp[:, :cw], in0=ps[:, :cw],
                                        in1=sh[:, :cw], op=OP.subtract)
                sl = wp.tile([1, CW], BF, tag="sl")
                nc.vector.tensor_copy(out=sl[:, :cw], in_=tmp[:, :cw])
                nc.sync.dma_start(out=shi[:, o:o + cw], in_=sh[:, :cw])
                nc.sync.dma_start(out=slo[:, o:o + cw], in_=sl[:, :cw])
                pd = pp1.tile([1, CW], F32, space="PSUM", tag="pdv")
                nc.tensor.matmul(out=pd[:, :cw], lhsT=adb[:],
                                 rhs=h1T[:, o:o + cw], start=True, stop=True)
                dv = wp.tile([1, CW], F32, tag="dv")
                nc.vector.tensor_copy(out=dv[:, :cw], in_=pd[:, :cw])
                nc.sync.dma_start(out=d1o[:, o:o + cw], in_=dv[:, :cw])
    nc.finalize()
    return nc


# --------------------------------------------------------- edge-phase NEFFs
def _build_edge_neff(N, CH, NW, NS, G, cumG, nslot, layer, FH, FO, Hnext):
    """layer 1: aggregates FH-dim messages, computes x2=relu(.+b1), h2/s2/d2.
       layer 2: aggregates FH-dim messages, emits sigmoid output [CH, FH].
       FH: feature dim of this layer's h.  FO: next-layer dim (layer 1 only).
    """
    TC = 384 if layer == 1 else 256
    SC = 2 * FH + 1                     # s_hi col (after h_hi, h_lo, ones)
    RC = 2 * FH + 1                     # rhs cols: h_hi | h_lo | ones
    WT = NW * WIN
    BW = WT + 128                       # padded width for B/D slices

    nc = bacc.Bacc(None, target_bir_lowering=False)
    T = nc.declare_dram_parameter("T", [N, TC], BF, isOutput=False)
    dloc = nc.declare_dram_parameter("dloc", [1, BW], F32, isOutput=False)
    iot = nc.declare_dram_parameter("iot", [128, 128], BF, isOutput=False)
    one1 = nc.declare_dram_parameter("one1", [1, 128], BF, isOutput=False)
    brep = nc.declare_dram_parameter("brep", [128, FH], F32, isOutput=False)
    gidx_d, dcol_d = [], []
    for s in range(NS):
        if nslot[s] == 0:
            gidx_d.append(None)
            dcol_d.append(None)
            continue
        gidx_d.append(nc.declare_dram_parameter(
            f"gidx{s}", [128, nslot[s] // 16], I16, isOutput=False))
        dcol_d.append(nc.declare_dram_parameter(
            f"dcol{s}", [128, nslot[s] // 128], BF, isOutput=False))
    if layer == 1:
        W2 = nc.declare_dram_parameter("W2", [FH, FO], F32, isOutput=False)
        a2s = nc.declare_dram_parameter("a2s", [FO, 1], F32, isOutput=False)
        a2d = nc.declare_dram_parameter("a2d", [FO, 1], F32, isOutput=False)
        hhi = nc.declare_dram_parameter("hhi", [FO, WT], BF, isOutput=True)
        hlo = nc.declare_dram_parameter("hlo", [FO, WT], BF, isOutput=True)
        shi = nc.declare_dram_parameter("shi", [1, WT], BF, isOutput=True)
        slo = nc.declare_dram_parameter("slo", [1, WT], BF, isOutput=True)
        d2o = nc.declare_dram_parameter("d2o", [1, WT], F32, isOutput=True)
    else:
        outp = nc.declare_dram_parameter("out", [CH, FH], F32, isOutput=True)

    # megatile group spans per set
    mts = []
    for wa in range(0, NW, MW):
        wb = min(wa + MW, NW)
        span = [(int(cumG[s][wa]), int(cumG[s][wb])) for s in range(NS)]
        mts.append((wa, wb, span))
    maxg = [max((b - a) for _, _, sp in mts for (a, b) in [sp[s]]) or 1
            for s in range(NS)]

    with tile.TileContext(nc) as tc:
        with tc.tile_pool(name="cst", bufs=1) as cp:
            iosb = cp.tile([128, 128], BF, tag="io")
            nc.sync.dma_start(out=iosb[:], in_=iot[:])
            onsb = cp.tile([1, 128], BF, tag="on")
            nc.sync.dma_start(out=onsb[:], in_=one1[:])
            bsb = cp.tile([128, FH], F32, tag="bs")
            nc.sync.dma_start(out=bsb[:], in_=brep[:])
            Bt = cp.tile([1, BW], BF, tag="Bt")
            Dt = cp.tile([1, BW], BF, tag="Dt")
            with tc.tile_pool(name="dtmp", bufs=1) as dtp:
                dsb = dtp.tile([1, BW], F32, tag="ds")
                nc.sync.dma_start(out=dsb[:], in_=dloc[:])
                nc.scalar.activation(Bt[:], dsb[:], AF.Exp)
                nc.scalar.activation(Dt[:], dsb[:], AF.Exp, scale=0.2)
            if layer == 1:
                idn = cp.tile([128, 128], F32, tag="idn")
                make_identity(nc, idn[:])
                x2T = cp.tile([128, WT], F32, tag="x2T")
                w2sb = cp.tile([FH, FO], F32, tag="w2")
                nc.sync.dma_start(out=w2sb[:], in_=W2[:])
                a2ssb = cp.tile([FO, 1], F32, tag="a2s")
                nc.sync.dma_start(out=a2ssb[:], in_=a2s[:])
                a2dsb = cp.tile([FO, 1], F32, tag="a2d")
                nc.sync.dma_start(out=a2dsb[:], in_=a2d[:])

            with tc.tile_pool(name="gth", bufs=2) as gp, \
                 tc.tile_pool(name="wk", bufs=4) as wp, \
                 tc.tile_pool(name="msk", bufs=4) as mp, \
                 tc.tile_pool(name="pm", bufs=2, space="PSUM") as pmp, \
                 tc.tile_pool(name="pb", bufs=2, space="PSUM") as pbp, \
                 tc.tile_pool(name="pt", bufs=2, space="PSUM") as ptp:
                for wa, wb, span in mts:
                    gts, Ats, Cts, dcs = [], [], [], []
                    for s in range(NS):
                        ga, gb = span[s]
                        if gb == ga:
                            gts.append(None)
                            Ats.append(None)
                            Cts.append(None)
                            dcs.append(None)
                            continue
                        gsp = gb - ga
                        ix = gp.tile([128, maxg[s] * 8], I16, tag=f"ix{s}")
                        nc.sync.dma_start(out=ix[:, :gsp * 8],
                                          in_=gidx_d[s][:, ga * 8:gb * 8])
                        gt = gp.tile([128, maxg[s], TC], BF, tag=f"gt{s}")
                        nc.gpsimd.dma_gather(
                            out_ap=gt[:, :gsp, :],
                            in_ap=T[s * SETROWS:, :],
                            idxs_ap=ix[:, :gsp * 8],
                            num_idxs=gsp * 128,
                            num_idxs_reg=gsp * 128,
                            elem_size=TC,
                            single_packet=False,
                        )
                        dc = wp.tile([128, maxg[s]], BF, tag=f"dc{s}")
                        nc.sync.dma_start(out=dc[:, :gsp],
                                          in_=dcol_d[s][:, ga:gb])
                        se = wp.tile([128, maxg[s]], F32, tag=f"se{s}")
                        nc.vector.tensor_tensor(out=se[:, :gsp],
                                                in0=gt[:, :gsp, SC],
                                                in1=gt[:, :gsp, SC + 1],
                                                op=OP.add)
                        At = wp.tile([128, maxg[s]], F32, tag=f"At{s}")
                        nc.scalar.activation(At[:, :gsp], se[:, :gsp], AF.Exp)
                        Ct = wp.tile([128, maxg[s]], F32, tag=f"Ct{s}")
                        nc.scalar.activation(Ct[:, :gsp], se[:, :gsp], AF.Exp,
                                             scale=0.2)
                        gts.append(gt)
                        Ats.append(At)
                        Cts.append(Ct)
                        dcs.append(dc)
                    for w in range(wa, wb):
                        ngrp = int(G[:, w].sum())
                        if ngrp == 0:
                            continue
                        w0 = w * WIN
                        pb = pbp.tile([128, 128], F32, space="PSUM", tag="pb")
                        nc.tensor.matmul(out=pb[:], lhsT=onsb[:],
                                         rhs=Bt[:, w0:w0 + 128],
                                         start=True, stop=True)
                        Br = mp.tile([128, 128], BF, tag="Br")
                        nc.vector.tensor_copy(out=Br[:], in_=pb[:])
                        pd2 = pbp.tile([128, 128], F32, space="PSUM", tag="pd2")
                        nc.tensor.matmul(out=pd2[:], lhsT=onsb[:],
                                         rhs=Dt[:, w0:w0 + 128],
                                         start=True, stop=True)
                        Dr = mp.tile([128, 128], BF, tag="Dr")
                        nc.vector.tensor_copy(out=Dr[:], in_=pd2[:])

                        psum = pmp.tile([128, RC], F32, space="PSUM", tag="ps")
                        gi = 0
                        for s in range(NS):
                            ga, _ = span[s]
                            for j in range(int(G[s][w])):
                                g = int(cumG[s][w]) - ga + j
                                gg = g
                                t2 = mp.tile([128, 128], BF, tag="t2")
                                nc.scalar.activation(
                                    t2[:], Dr[:], AF.Copy,
                                    scale=Cts[s][:, gg:gg + 1])
                                t1 = mp.tile([128, 128], BF, tag="t1")
                                nc.vector.scalar_tensor_tensor(
                                    out=t1[:], in0=Br[:],
                                    scalar=Ats[s][:, gg:gg + 1], in1=t2[:],
                                    op0=OP.mult, op1=OP.max)
                                sal = mp.tile([128, 128], BF, tag="sal")
                                nc.vector.scalar_tensor_tensor(
                                    out=sal[:], in0=iosb[:],
                                    scalar=dcs[s][:, gg:gg + 1], in1=t1[:],
                                    op0=OP.is_equal, op1=OP.mult)
                                nc.tensor.matmul(
                                    out=psum[:], lhsT=sal[:],
                                    rhs=gts[s][:, g, 0:RC],
                                    start=(gi == 0), stop=(gi == ngrp - 1))
                                gi += 1
                        # ---- window tail
                        pc = wp.tile([128, RC], F32, tag="pc")
                        nc.vector.tensor_copy(out=pc[:], in_=psum[:])
                        u = wp.tile([128, FH], F32, tag="u")
                        nc.vector.tensor_tensor(out=u[:], in0=pc[:, 0:FH],
                                                in1=pc[:, FH:2 * FH],
                                                op=OP.add)
                        zeps = wp.tile([128, 1], F32, tag="zeps")
                        nc.vector.tensor_scalar(
                            out=zeps[:], in0=pc[:, 2 * FH:2 * FH + 1],
                            scalar1=1e-16, scalar2=None, op0=OP.add)
                        rz = wp.tile([128, 1], F32, tag="rz")
                        nc.vector.reciprocal(out=rz[:], in_=zeps[:])
                        o1 = wp.tile([128, FH], F32, tag="o1")
                        nc.vector.tensor_scalar(
                            out=o1[:], in0=u[:], scalar1=rz[:], scalar2=None,
                            op0=OP.mult)
                        xb = wp.tile([128, FH], F32, tag="xb")
                        nc.vector.tensor_tensor(out=xb[:], in0=o1[:],
                                                in1=bsb[:], op=OP.add)
                        nr = min(WIN, CH - w0)
                        if layer == 1:
                            x2 = wp.tile([128, FH], F32, tag="x2")
                            nc.vector.tensor_scalar(
                                out=x2[:], in0=xb[:], scalar1=0.0,
                                scalar2=None, op0=OP.max)
                            pt = ptp.tile([128, 128], F32, space="PSUM",
                                          tag="pt")
                            nc.tensor.transpose(pt[:], x2[:], idn[:])
                            nc.vector.tensor_copy(out=x2T[:, w0:w0 + WIN],
                                                  in_=pt[:, 0:WIN])
                        else:
                            sg = wp.tile([128, FH], F32, tag="sg")
                            nc.scalar.activation(sg[:], xb[:], AF.Sigmoid)
                            nc.sync.dma_start(out=outp[w0:w0 + nr, :],
                                              in_=sg[0:nr, :])

            if layer == 1:
                with tc.tile_pool(name="tl", bufs=3) as tp, \
                     tc.tile_pool(name="tc1", bufs=1) as tcp, \
                     tc.tile_pool(name="ph2", bufs=2, space="PSUM") as php, \
                     tc.tile_pool(name="psv", bufs=2, space="PSUM") as psp:
                    h2T = tcp.tile([FO, WT], F32, tag="h2T")
                    CW = 512
                    for o in range(0, WT, CW):
                        cw = min(CW, WT - o)
                        ph = php.tile([FO, CW], F32, space="PSUM", tag="ph")
                        nc.tensor.matmul(out=ph[:, :cw], lhsT=w2sb[:],
                                         rhs=x2T[:, o:o + cw],
                                         start=True, stop=True)
                        nc.vector.tensor_copy(out=h2T[:, o:o + cw], in_=ph[:, :cw])
                        hh = tp.tile([FO, CW], BF, tag="hh")
                        nc.scalar.activation(hh[:, :cw], ph[:, :cw], AF.Copy)
                        tmp = tp.tile([FO, CW], F32, tag="tmp")
                        nc.vector.tensor_tensor(out=tmp[:, :cw], in0=ph[:, :cw],
                                                in1=hh[:, :cw], op=OP.subtract)
                        hl = tp.tile([FO, CW], BF, tag="hl")
                        nc.vector.tensor_copy(out=hl[:, :cw], in_=tmp[:, :cw])
                        nc.sync.dma_start(out=hhi[:, o:o + cw], in_=hh[:, :cw])
                        nc.sync.dma_start(out=hlo[:, o:o + cw], in_=hl[:, :cw])
                    for o in range(0, WT, CW):
                        cw = min(CW, WT - o)
                        ps = psp.tile([1, CW], F32, space="PSUM", tag="ps2")
                        nc.tensor.matmul(out=ps[:, :cw], lhsT=a2ssb[:],
                                         rhs=h2T[:, o:o + cw],
                                         start=True, stop=True)
                        sh = tp.tile([1, CW], BF, tag="sh")
                        nc.scalar.activation(sh[:, :cw], ps[:, :cw], AF.Copy)
                        tmp = tp.tile([1, CW], F32, tag="tms")
                        nc.vector.tensor_tensor(out=tmp[:, :cw], in0=ps[:, :cw],
                                                in1=sh[:, :cw], op=OP.subtract)
                        sl = tp.tile([1, CW], BF, tag="sl")
                        nc.vector.tensor_copy(out=sl[:, :cw], in_=tmp[:, :cw])
                        nc.sync.dma_start(out=shi[:, o:o + cw], in_=sh[:, :cw])
                        nc.sync.dma_start(out=slo[:, o:o + cw], in_=sl[:, :cw])
                        pd = psp.tile([1, CW], F32, space="PSUM", tag="pd")
                        nc.tensor.matmul(out=pd[:, :cw], lhsT=a2dsb[:],
                                         rhs=h2T[:, o:o + cw],
                                         start=True, stop=True)
                        dv = tp.tile([1, CW], F32, tag="dv")
                        nc.vector.tensor_copy(out=dv[:, :cw], in_=pd[:, :cw])
                        nc.sync.dma_start(out=d2o[:, o:o + cw], in_=dv[:, :cw])
    nc.finalize()
    return nc


# ------------------------------------------------------------------- driver
def kernel(edge_index, embed, W1, a_src1, a_dst1, b1, W2, a_src2, a_dst2, b2):
    N, C = embed.shape
    H = W1.shape[1]
    K = W2.shape[1]
    CH = N // NCORES
    meta = _preprocess(np.asarray(edge_index), N)
    NW, NS, G, cumG, nslot = (meta['NW'], meta['NS'], meta['G'],
                              meta['cumG'], meta['nslot'])
    WT = NW * WIN
    BW = WT + 128
    cores = list(range(NCORES))

    # ---- NEFF 1
    nc1 = _build_neff1(N, C, H, CH)
    maps1 = []
    for c in range(NCORES):
        xt = np.ascontiguousarray(embed[c * CH:(c + 1) * CH, :].T)
        maps1.append({"xT": xt.astype(np.float32),
                      "W1": np.asarray(W1, np.float32),
                      "a1s": np.asarray(a_src1, np.float32)[:, None],
                      "a1d": np.asarray(a_dst1, np.float32)[:, None]})
    print("[kernel] NEFF1 built, running...", file=sys.stderr, flush=True)
    res1 = run_bass_kernel_spmd(nc1, maps1, cores, **_trace_kw("n1"))
    r1 = res1.results
    _record(res1)
    print("[kernel] NEFF1 done", file=sys.stderr, flush=True)

    T1 = np.zeros((N, 384), BF16)
    d1 = np.zeros((NCORES, 1, BW), np.float32)
    for c in range(NCORES):
        sl = slice(c * CH, (c + 1) * CH)
        T1[sl, 0:H] = r1[c]["hhi"].T
        T1[sl, H:2 * H] = r1[c]["hlo"].T
        T1[sl, 2 * H] = BF16(1.0)
        T1[sl, 2 * H + 1] = r1[c]["shi"][0]
        T1[sl, 2 * H + 2] = r1[c]["slo"][0]
        d1[c, 0, :CH] = r1[c]["d1o"][0]

    iota_np = np.tile(np.arange(128, dtype=np.float32), (128, 1)).astype(BF16)
    ones_np = np.ones((1, 128), BF16)

    # ---- NEFF 2
    nc2 = _build_edge_neff(N, CH, NW, NS, G, cumG, nslot, 1, H, K, None)
    maps2 = []
    for c in range(NCORES):
        m = {"T": T1, "dloc": d1[c], "iot": iota_np, "one1": ones_np,
             "brep": np.tile(np.asarray(b1, np.float32), (128, 1)),
             "W2": np.asarray(W2, np.float32),
             "a2s": np.asarray(a_src2, np.float32)[:, None],
             "a2d": np.asarray(a_dst2, np.float32)[:, None]}
        for s in range(NS):
            if nslot[s] == 0:
                continue
            m[f"gidx{s}"] = meta['cores'][c][0][s]
            m[f"dcol{s}"] = meta['cores'][c][1][s]
        maps2.append(m)
    print("[kernel] NEFF2 built, running...", file=sys.stderr, flush=True)
    res2 = run_bass_kernel_spmd(nc2, maps2, cores, **_trace_kw("n2"))
    r2 = res2.results
    _record(res2)
    print("[kernel] NEFF2 done", file=sys.stderr, flush=True)

    T2 = np.zeros((N, 256), BF16)
    d2 = np.zeros((NCORES, 1, BW), np.float32)
    for c in range(NCORES):
        sl = slice(c * CH, (c + 1) * CH)
        T2[sl, 0:K] = r2[c]["hhi"][:, :CH].T
        T2[sl, K:2 * K] = r2[c]["hlo"][:, :CH].T
        T2[sl, 2 * K] = BF16(1.0)
        T2[sl, 2 * K + 1] = r2[c]["shi"][0, :CH]
        T2[sl, 2 * K + 2] = r2[c]["slo"][0, :CH]
        d2[c, 0, :CH] = r2[c]["d2o"][0, :CH]

    # ---- NEFF 3
    nc3 = _build_edge_neff(N, CH, NW, NS, G, cumG, nslot, 2, K, None, None)
    maps3 = []
    for c in range(NCORES):
        m = {"T": T2, "dloc": d2[c], "iot": iota_np, "one1": ones_np,
             "brep": np.tile(np.asarray(b2, np.float32), (128, 1))}
        for s in range(NS):
            if nslot[s] == 0:
                continue
            m[f"gidx{s}"] = meta['cores'][c][0][s]
            m[f"dcol{s}"] = meta['cores'][c][1][s]
        maps3.append(m)
    print("[kernel] NEFF3 built, running...", file=sys.stderr, flush=True)
    res3 = run_bass_kernel_spmd(nc3, maps3, cores, **_trace_kw("n3"))
    r3 = res3.results
    _record(res3)
    print("[kernel] NEFF3 done", file=sys.stderr, flush=True)

    out = np.concatenate([r3[c]["out"] for c in range(NCORES)], axis=0)
    return out.astype(np.float32)

